# revision 37
# baseline (speedup 1.0000x reference)
"""HQQ 4-bit quantized linear on 8 Trainium2 NeuronCores (Bass/Tile).

out[4096, 11008] = x[4096, 4096] @ dequant(W_q, scale, zero).T + bias

Index fact: reference reshapes ((W_r - zero) * scale) from [64, 704512] to
[11008, 4096].  With o = output feature, i = input feature:
    o = g_row * 172 + j,   group g = j * 4096 + i,   g_row in [0, 64)
so 8 consecutive g_rows per core give a contiguous 1376-column output slice
(column-parallel linear, x replicated).

Design (measured ~646 us vs 1287 us for the transpose-on-device baseline;
PE streams back-to-back at 216 ns per N=512 matmul, ~91% of bf16 peak):
  * host prep (layout/precision only): x transposed to [i, t] and cast to
    bf16 (the same rounding the device would apply); per-core W_q slice
    repacked to uint16 [i, o_c] (u16 so every DVE dequant op runs the
    2x 16-bit path); scale/zero transposed+bf16 in a [p, kb*J+j] layout
    that loads with one contiguous DMA each.
  * device dequant happens directly in the transposed layout -> NO PE
    transposes, all DVE tiles full 128 partitions:
        nib = (q >> sh) & 15        (sh = 4 on hi-nibble cores, 0 on lo)
        W^T[:, k, (r,j)] = (nib - zero[j]) * scale[j]   (broadcast over r)
  * GEMM keeps W^T stationary in the PE (FWL-eligible bf16 128-col loads,
    LDWEIGHTS fully hidden) and streams x^T:
        out_T[o, t] = sum_k WT[k,o].T @ xh[k,t],  N = 512 (one PSUM bank).
  * token chunk 0 runs k-major across 8 o-tiles (8 PSUM banks) so the
    matmuls start as soon as dequant emits each k-slice; dequant is split
    by output rows: pass A (r 0..5, covers those 8 o-tiles) paces the
    warmup at ~1.5 us/k, pass B (r 6..7) runs later under full-rate GEMM
    when the DVE is otherwise idle.  Remaining chunks run o-tile-major.
  * bias is added during the PSUM->SBUF drain (DVE tensor_scalar add).
Output is written o-major [1376, 4096] per core; host transposes back.
"""

import numpy as np
from contextlib import ExitStack

import concourse.bacc as bacc
import concourse.bass as bass
import concourse.mybir as mybir
import concourse.tile as tile
from concourse.bass_utils import run_bass_kernel_spmd

dt = mybir.dt
Alu = mybir.AluOpType

TOKENS, IN_F, OUT_F, GS = 4096, 4096, 11008, 64
G = OUT_F * IN_F // GS            # 704512 quantization groups
J = G // IN_F                     # 172 groups per (g_row, i) plane
NCORES = 8
RPC = GS // NCORES                # 8 g_rows per core
O_C = RPC * J                     # 1376 output cols per core
NK = IN_F // 128                  # 32 contraction blocks
TC = 512                          # token chunk (= max matmul N)
NTC = TOKENS // TC                # 8 token chunks
NOT = (O_C + 127) // 128          # 11 o-tiles (last one 96 rows)
KMAJ = 8                          # o-tiles run k-major during chunk 0

_CACHE = {}


def _build():
    nc = bacc.Bacc("TRN2", target_bir_lowering=False, debug=False,
                   num_devices=NCORES)

    xt_d = nc.dram_tensor("xt", [IN_F, TOKENS], dt.bfloat16, kind="ExternalInput")
    q_d = nc.dram_tensor("qt", [IN_F, O_C], dt.uint16, kind="ExternalInput")
    s_d = nc.dram_tensor("st", [128, NK * J], dt.bfloat16, kind="ExternalInput")
    z_d = nc.dram_tensor("zt", [128, NK * J], dt.bfloat16, kind="ExternalInput")
    b_d = nc.dram_tensor("biasc", [128, NOT], dt.float32, kind="ExternalInput")
    sh_d = nc.dram_tensor("shc", [128, 1], dt.int32, kind="ExternalInput")
    o_d = nc.dram_tensor("out", [O_C, TOKENS], dt.float32, kind="ExternalOutput")

    with ExitStack() as ctx:
        tc_ = ctx.enter_context(tile.TileContext(nc))
        const = ctx.enter_context(tc_.tile_pool(name="const", bufs=1))
        wpool = ctx.enter_context(tc_.tile_pool(name="wt", bufs=1))
        dq = ctx.enter_context(tc_.tile_pool(name="dq", bufs=2))
        xh_p = ctx.enter_context(tc_.tile_pool(name="xh", bufs=2))
        ob_p = ctx.enter_context(tc_.tile_pool(name="ob", bufs=4))
        pp = ctx.enter_context(
            tc_.tile_pool(name="pp", bufs=8, space=bass.MemorySpace.PSUM))

        shc = const.tile([128, 1], dt.int32)
        nc.sync.dma_start(shc[:], sh_d[:])
        # scale/zero resident, one contiguous DMA each: [p, kb*J + j].
        # (DMAs for these are emitted inside the first dequant call, after
        # the first q tile's, so the critical first nib isn't queued behind
        # two 350 KB transfers.)
        z_all = const.tile([128, NK * J], dt.bfloat16)
        s_all = const.tile([128, NK * J], dt.bfloat16)
        biasc = const.tile([128, NOT], dt.float32)

        # resident dequantized transposed weights: [i-in-block, k-block, o]
        WT = wpool.tile([128, NK, O_C], dt.bfloat16)

        consts_loaded = [False]

        def dequant(k, r0, r1):
            """Dequantize WT[:, k, r0*J:r1*J] (r-aligned o-range)."""
            nr = r1 - r0
            o0, o1 = r0 * J, r1 * J
            q = dq.tile([128, nr * J], dt.uint16, tag=f"q{r0}")
            nc.sync.dma_start(q[:], q_d[k * 128:(k + 1) * 128, o0:o1])
            if not consts_loaded[0]:
                consts_loaded[0] = True
                nc.sync.dma_start(z_all[:], z_d[:])
                nc.sync.dma_start(s_all[:], s_d[:])
                nc.sync.dma_start(biasc[:], b_d[:])
            zk = z_all[:, k * J:(k + 1) * J]
            sk = s_all[:, k * J:(k + 1) * J]
            nib = dq.tile([128, nr * J], dt.uint16, tag=f"nib{r0}")
            nc.vector.tensor_scalar(nib[:], q[:], shc[:, 0:1], 15,
                                    Alu.logical_shift_right, Alu.bitwise_and)
            tmp = dq.tile([128, nr, J], dt.bfloat16, tag=f"tmp{r0}")
            nc.vector.tensor_tensor(
                tmp[:], nib[:].rearrange("p (r j) -> p r j", r=nr),
                zk[:, None, :].to_broadcast([128, nr, J]), Alu.subtract)
            wv = WT[:, k, o0:o1].rearrange("p (r j) -> p r j", r=nr)
            if nr == RA and k >= 2:
                # split the scale multiply so pass A paces under the PE's
                # 1.7 us/k: DVE does r0..3, the otherwise-idle GPSIMD r4..5
                nd = nr - 2
                nc.vector.tensor_tensor(
                    wv[:, 0:nd, :], tmp[:, 0:nd, :],
                    sk[:, None, :].to_broadcast([128, nd, J]), Alu.mult)
                nc.gpsimd.tensor_tensor(
                    wv[:, nd:nr, :], tmp[:, nd:nr, :],
                    sk[:, None, :].to_broadcast([128, 2, J]), Alu.mult)
            else:
                nc.vector.tensor_tensor(
                    wv[:], tmp[:],
                    sk[:, None, :].to_broadcast([128, nr, J]), Alu.mult)

        def x_chunk_tile(xh, t, k):
            nc.sync.dma_start(
                xh[:, k, :], xt_d[k * 128:(k + 1) * 128, t * TC:(t + 1) * TC])

        def o_rows(ot):
            return min(128, O_C - ot * 128)

        def drain(acc, ot, t):
            m = o_rows(ot)
            ob = ob_p.tile([128, TC], dt.float32, tag="ob")
            nc.vector.tensor_scalar_add(ob[0:m, :], acc[0:m, :],
                                        biasc[0:m, ot:ot + 1])
            nc.sync.dma_start(
                o_d[ot * 128:ot * 128 + m, t * TC:(t + 1) * TC], ob[0:m, :])

        # ---- token chunk 0: k-major over the first 8 o-tiles, interleaved
        # with dequant pass A (r 0..5, covers o < 1032 > 8*128) so the PE
        # starts as soon as each WT[:, k, :1032] slice lands.  Pass B
        # (r 6..7) is emitted afterwards; its consumers (o-tiles 8..10)
        # run while the PE is already at full rate. ----
        RA = 6
        xh0 = xh_p.tile([128, NK, TC], dt.bfloat16, tag="xh")
        accs = [pp.tile([128, TC], dt.float32, tag="acc", name=f"acc{i}")
                for i in range(KMAJ)]
        def ot_unit(xh, t, ot):
            m = o_rows(ot)
            acc = pp.tile([128, TC], dt.float32, tag="acc", name="acc")
            for k in range(NK):
                nc.tensor.matmul(
                    acc[0:m, :], WT[:, k, ot * 128:ot * 128 + m],
                    xh[:, k, :], start=(k == 0), stop=(k == NK - 1))
            drain(acc, ot, t)

        for k in range(NK):
            if k == 0:
                # smaller first chunks -> first matmul starts sooner
                for r in range(0, RA, 2):
                    dequant(0, r, r + 2)
            else:
                dequant(k, 0, RA)
            x_chunk_tile(xh0, 0, k)
            for ot in range(KMAJ):
                nc.tensor.matmul(
                    accs[ot][:], WT[:, k, ot * 128:(ot + 1) * 128],
                    xh0[:, k, :], start=(k == 0), stop=(k == NK - 1))
        for ot in range(KMAJ):
            drain(accs[ot], ot, 0)

        # tc1 o-tiles 0..7 need only pass A; they keep the PE at full rate
        # while pass B dequantizes r 6..7 on the otherwise-idle DVE.
        xh1 = xh_p.tile([128, NK, TC], dt.bfloat16, tag="xh")
        for k in range(NK):
            x_chunk_tile(xh1, 1, k)
        for ot in range(KMAJ):
            ot_unit(xh1, 1, ot)
        for k in range(NK):
            dequant(k, RA, RPC)
        for ot in range(KMAJ, NOT):
            ot_unit(xh0, 0, ot)
        for ot in range(KMAJ, NOT):
            ot_unit(xh1, 1, ot)

        # ---- token chunks 2..7: o-tile-major at full PE rate ----
        for t in range(2, NTC):
            xh = xh_p.tile([128, NK, TC], dt.bfloat16, tag="xh")
            for k in range(NK):
                x_chunk_tile(xh, t, k)
            for ot in range(NOT):
                ot_unit(xh, t, ot)

    nc.compile()
    return nc


def get_nc():
    if "nc" not in _CACHE:
        _CACHE["nc"] = _build()
    return _CACHE["nc"]


def make_in_maps(x, W_q, scale, zero, bias):
    import ml_dtypes
    x = np.asarray(x, dtype=np.float32)
    W_q = np.asarray(W_q, dtype=np.int32)
    xt = np.ascontiguousarray(x.T.astype(ml_dtypes.bfloat16))    # [i, t]

    def kj_layout(v):
        # [J, IN_F] params -> [p, kb*J + j] so one contiguous DMA loads all
        t = np.asarray(v, dtype=np.float32).reshape(J, IN_F).T   # [i, j]
        t = t.reshape(NK, 128, J).transpose(1, 0, 2).reshape(128, NK * J)
        return np.ascontiguousarray(t.astype(ml_dtypes.bfloat16))

    st = kj_layout(scale)
    zt = kj_layout(zero)
    bias = np.asarray(bias, dtype=np.float32)
    in_maps = []
    for c in range(NCORES):
        r0 = RPC * (c % 4)
        qt = np.ascontiguousarray(
            W_q[r0:r0 + RPC].reshape(RPC, J, IN_F).astype(np.uint16)
            .transpose(2, 0, 1).reshape(IN_F, O_C))          # [i, r*J+j]
        bpad = np.zeros(NOT * 128, dtype=np.float32)
        bpad[:O_C] = bias[c * O_C:(c + 1) * O_C]
        in_maps.append({
            "xt": xt,
            "qt": qt,
            "st": st,
            "zt": zt,
            "biasc": np.ascontiguousarray(bpad.reshape(NOT, 128).T),
            "shc": np.full((128, 1), 4 if c < 4 else 0, dtype=np.int32),
        })
    return in_maps


def assemble(results):
    out = np.empty((TOKENS, OUT_F), dtype=np.float32)
    for c in range(NCORES):
        out[:, c * O_C:(c + 1) * O_C] = results[c]["out"].T
    return out


def kernel(x, W_q, scale, zero, bias):
    nc = get_nc()
    in_maps = make_in_maps(x, W_q, scale, zero, bias)
    res = run_bass_kernel_spmd(nc, in_maps, list(range(NCORES)))
    return assemble(res.results)


# revision 38
# speedup vs baseline: 1.0110x; 1.0110x over previous
"""HQQ 4-bit quantized linear on 8 Trainium2 NeuronCores (Bass/Tile).

out[4096, 11008] = x[4096, 4096] @ dequant(W_q, scale, zero).T + bias

Index fact: reference reshapes ((W_r - zero) * scale) from [64, 704512] to
[11008, 4096].  With o = output feature, i = input feature:
    o = g_row * 172 + j,   group g = j * 4096 + i,   g_row in [0, 64)
so 8 consecutive g_rows per core give a contiguous 1376-column output slice
(column-parallel linear, x replicated).

Design (measured ~646 us vs 1287 us for the transpose-on-device baseline;
PE streams back-to-back at 216 ns per N=512 matmul, ~91% of bf16 peak):
  * host prep (layout/precision only): x transposed to [i, t] and cast to
    bf16 (the same rounding the device would apply); per-core W_q slice
    repacked to uint16 [i, o_c] (u16 so every DVE dequant op runs the
    2x 16-bit path); scale/zero transposed+bf16 in a [p, kb*J+j] layout
    that loads with one contiguous DMA each.
  * device dequant happens directly in the transposed layout -> NO PE
    transposes, all DVE tiles full 128 partitions:
        nib = (q >> sh) & 15        (sh = 4 on hi-nibble cores, 0 on lo)
        W^T[:, k, (r,j)] = (nib - zero[j]) * scale[j]   (broadcast over r)
  * GEMM keeps W^T stationary in the PE (FWL-eligible bf16 128-col loads,
    LDWEIGHTS fully hidden) and streams x^T:
        out_T[o, t] = sum_k WT[k,o].T @ xh[k,t],  N = 512 (one PSUM bank).
  * token chunk 0 runs k-major across 8 o-tiles (8 PSUM banks) so the
    matmuls start as soon as dequant emits each k-slice; dequant is split
    by output rows: pass A (r 0..5, covers those 8 o-tiles) paces the
    warmup at ~1.5 us/k, pass B (r 6..7) runs later under full-rate GEMM
    when the DVE is otherwise idle.  Remaining chunks run o-tile-major.
  * bias is added during the PSUM->SBUF drain (DVE tensor_scalar add).
Output is written o-major [1376, 4096] per core; host transposes back.
"""

import numpy as np
from contextlib import ExitStack

import concourse.bacc as bacc
import concourse.bass as bass
import concourse.mybir as mybir
import concourse.tile as tile
from concourse.bass_utils import run_bass_kernel_spmd

dt = mybir.dt
Alu = mybir.AluOpType

TOKENS, IN_F, OUT_F, GS = 4096, 4096, 11008, 64
G = OUT_F * IN_F // GS            # 704512 quantization groups
J = G // IN_F                     # 172 groups per (g_row, i) plane
NCORES = 8
RPC = GS // NCORES                # 8 g_rows per core
O_C = RPC * J                     # 1376 output cols per core
NK = IN_F // 128                  # 32 contraction blocks
TC = 512                          # token chunk (= max matmul N)
NTC = TOKENS // TC                # 8 token chunks
NOT = (O_C + 127) // 128          # 11 o-tiles (last one 96 rows)
KMAJ = 8                          # o-tiles run k-major during chunk 0

_CACHE = {}


def _build():
    nc = bacc.Bacc("TRN2", target_bir_lowering=False, debug=False,
                   num_devices=NCORES)

    xt_d = nc.dram_tensor("xt", [IN_F, TOKENS], dt.bfloat16, kind="ExternalInput")
    q_d = nc.dram_tensor("qt", [IN_F, O_C], dt.uint16, kind="ExternalInput")
    s_d = nc.dram_tensor("st", [128, NK * J], dt.bfloat16, kind="ExternalInput")
    z_d = nc.dram_tensor("zt", [128, NK * J], dt.bfloat16, kind="ExternalInput")
    b_d = nc.dram_tensor("biasc", [128, NOT], dt.float32, kind="ExternalInput")
    sh_d = nc.dram_tensor("shc", [128, 1], dt.int32, kind="ExternalInput")
    o_d = nc.dram_tensor("out", [O_C, TOKENS], dt.float32, kind="ExternalOutput")

    with ExitStack() as ctx:
        tc_ = ctx.enter_context(tile.TileContext(nc))
        const = ctx.enter_context(tc_.tile_pool(name="const", bufs=1))
        wpool = ctx.enter_context(tc_.tile_pool(name="wt", bufs=1))
        dq = ctx.enter_context(tc_.tile_pool(name="dq", bufs=2))
        xh_p = ctx.enter_context(tc_.tile_pool(name="xh", bufs=2))
        ob_p = ctx.enter_context(tc_.tile_pool(name="ob", bufs=4))
        pp = ctx.enter_context(
            tc_.tile_pool(name="pp", bufs=8, space=bass.MemorySpace.PSUM))

        shc = const.tile([128, 1], dt.int32)
        nc.sync.dma_start(shc[:], sh_d[:])
        # scale/zero resident, one contiguous DMA each: [p, kb*J + j].
        # (DMAs for these are emitted inside the first dequant call, after
        # the first q tile's, so the critical first nib isn't queued behind
        # two 350 KB transfers.)
        z_all = const.tile([128, NK * J], dt.bfloat16)
        s_all = const.tile([128, NK * J], dt.bfloat16)
        biasc = const.tile([128, NOT], dt.float32)

        # resident dequantized transposed weights: [i-in-block, k-block, o]
        WT = wpool.tile([128, NK, O_C], dt.bfloat16)

        consts_loaded = [False]

        def dequant(k, r0, r1):
            """Dequantize WT[:, k, r0*J:r1*J] (r-aligned o-range)."""
            nr = r1 - r0
            o0, o1 = r0 * J, r1 * J
            q = dq.tile([128, nr * J], dt.uint16, tag=f"q{r0}")
            nc.sync.dma_start(q[:], q_d[k * 128:(k + 1) * 128, o0:o1])
            if not consts_loaded[0]:
                consts_loaded[0] = True
                nc.sync.dma_start(z_all[:], z_d[:])
                nc.sync.dma_start(s_all[:], s_d[:])
                nc.sync.dma_start(biasc[:], b_d[:])
            zk = z_all[:, k * J:(k + 1) * J]
            sk = s_all[:, k * J:(k + 1) * J]
            nib = dq.tile([128, nr * J], dt.uint16, tag=f"nib{r0}")
            nc.vector.tensor_scalar(nib[:], q[:], shc[:, 0:1], 15,
                                    Alu.logical_shift_right, Alu.bitwise_and)
            tmp = dq.tile([128, nr, J], dt.bfloat16, tag=f"tmp{r0}")
            nc.vector.tensor_tensor(
                tmp[:], nib[:].rearrange("p (r j) -> p r j", r=nr),
                zk[:, None, :].to_broadcast([128, nr, J]), Alu.subtract)
            nc.vector.tensor_tensor(
                WT[:, k, o0:o1].rearrange("p (r j) -> p r j", r=nr), tmp[:],
                sk[:, None, :].to_broadcast([128, nr, J]), Alu.mult)

        def x_chunk_tile(xh, t, k):
            nc.sync.dma_start(
                xh[:, k, :], xt_d[k * 128:(k + 1) * 128, t * TC:(t + 1) * TC])

        def o_rows(ot):
            return min(128, O_C - ot * 128)

        def drain(acc, ot, t):
            m = o_rows(ot)
            ob = ob_p.tile([128, TC], dt.float32, tag="ob")
            nc.vector.tensor_scalar_add(ob[0:m, :], acc[0:m, :],
                                        biasc[0:m, ot:ot + 1])
            nc.sync.dma_start(
                o_d[ot * 128:ot * 128 + m, t * TC:(t + 1) * TC], ob[0:m, :])

        # ---- token chunk 0: k-major over the first 8 o-tiles, interleaved
        # with dequant pass A (r 0..5, covers o < 1032 > 8*128) so the PE
        # starts as soon as each WT[:, k, :1032] slice lands.  Pass B
        # (r 6..7) is emitted afterwards; its consumers (o-tiles 8..10)
        # run while the PE is already at full rate. ----
        RA = 6
        xh0 = xh_p.tile([128, NK, TC], dt.bfloat16, tag="xh")
        accs = [pp.tile([128, TC], dt.float32, tag="acc", name=f"acc{i}")
                for i in range(KMAJ)]
        def ot_unit(xh, t, ot):
            m = o_rows(ot)
            acc = pp.tile([128, TC], dt.float32, tag="acc", name="acc")
            for k in range(NK):
                nc.tensor.matmul(
                    acc[0:m, :], WT[:, k, ot * 128:ot * 128 + m],
                    xh[:, k, :], start=(k == 0), stop=(k == NK - 1))
            drain(acc, ot, t)

        for k in range(NK):
            if k == 0:
                # smaller first chunks -> first matmul starts sooner
                for r in range(0, RA, 2):
                    dequant(0, r, r + 2)
            else:
                dequant(k, 0, RA)
            x_chunk_tile(xh0, 0, k)
            for ot in range(KMAJ):
                nc.tensor.matmul(
                    accs[ot][:], WT[:, k, ot * 128:(ot + 1) * 128],
                    xh0[:, k, :], start=(k == 0), stop=(k == NK - 1))
        for ot in range(KMAJ):
            drain(accs[ot], ot, 0)

        # tc1 o-tiles 0..7 need only pass A; they keep the PE at full rate
        # while pass B dequantizes r 6..7 on the otherwise-idle DVE.
        xh1 = xh_p.tile([128, NK, TC], dt.bfloat16, tag="xh")
        for k in range(NK):
            x_chunk_tile(xh1, 1, k)
        for ot in range(KMAJ):
            ot_unit(xh1, 1, ot)
        for k in range(NK):
            dequant(k, RA, RPC)
        for ot in range(KMAJ, NOT):
            ot_unit(xh0, 0, ot)
        for ot in range(KMAJ, NOT):
            ot_unit(xh1, 1, ot)

        # ---- token chunks 2..7: o-tile-major at full PE rate ----
        for t in range(2, NTC):
            xh = xh_p.tile([128, NK, TC], dt.bfloat16, tag="xh")
            for k in range(NK):
                x_chunk_tile(xh, t, k)
            for ot in range(NOT):
                ot_unit(xh, t, ot)

    nc.compile()
    return nc


def get_nc():
    if "nc" not in _CACHE:
        _CACHE["nc"] = _build()
    return _CACHE["nc"]


def make_in_maps(x, W_q, scale, zero, bias):
    import ml_dtypes
    x = np.asarray(x, dtype=np.float32)
    W_q = np.asarray(W_q, dtype=np.int32)
    xt = np.ascontiguousarray(x.T.astype(ml_dtypes.bfloat16))    # [i, t]

    def kj_layout(v):
        # [J, IN_F] params -> [p, kb*J + j] so one contiguous DMA loads all
        t = np.asarray(v, dtype=np.float32).reshape(J, IN_F).T   # [i, j]
        t = t.reshape(NK, 128, J).transpose(1, 0, 2).reshape(128, NK * J)
        return np.ascontiguousarray(t.astype(ml_dtypes.bfloat16))

    st = kj_layout(scale)
    zt = kj_layout(zero)
    bias = np.asarray(bias, dtype=np.float32)
    in_maps = []
    for c in range(NCORES):
        r0 = RPC * (c % 4)
        qt = np.ascontiguousarray(
            W_q[r0:r0 + RPC].reshape(RPC, J, IN_F).astype(np.uint16)
            .transpose(2, 0, 1).reshape(IN_F, O_C))          # [i, r*J+j]
        bpad = np.zeros(NOT * 128, dtype=np.float32)
        bpad[:O_C] = bias[c * O_C:(c + 1) * O_C]
        in_maps.append({
            "xt": xt,
            "qt": qt,
            "st": st,
            "zt": zt,
            "biasc": np.ascontiguousarray(bpad.reshape(NOT, 128).T),
            "shc": np.full((128, 1), 4 if c < 4 else 0, dtype=np.int32),
        })
    return in_maps


def assemble(results):
    out = np.empty((TOKENS, OUT_F), dtype=np.float32)
    for c in range(NCORES):
        out[:, c * O_C:(c + 1) * O_C] = results[c]["out"].T
    return out


def kernel(x, W_q, scale, zero, bias):
    nc = get_nc()
    in_maps = make_in_maps(x, W_q, scale, zero, bias)
    res = run_bass_kernel_spmd(nc, in_maps, list(range(NCORES)))
    return assemble(res.results)
